# revision 1
# baseline (speedup 1.0000x reference)
"""Sharded cosine-similarity kNN (k=5) + weighted centroid on 8 TRN2 NeuronCores.

Strategy (standard sharded kNN):
  - Shard the 500000x768 f32 database row-wise across 8 cores (62500 rows each,
    padded to 62592 = 122 tiles x 512 rows + one 128-row tail tile, padding
    with copies of -query, whose cosine similarity is exactly -1 and can never
    enter the top-k).
  - Each core streams its ~192 MB shard from HBM once (memory-bound roofline,
    ~358 GB/s per core). Per [128, 3072] tile (4 db rows per partition,
    contiguous DMA), for each of the 4 row-groups:
      * DVE scalar_tensor_tensor (bypass,mult,accum): dot(row, q) fused
        multiply + free-dim sum
      * ACT activation(Square, accum_out): ||row||^2 fused square + sum
    (DVE ~468 us and ACT ~456 us busy hide under the ~540 us DMA stream.)
  - Epilogue per core: dn=sqrt(n2), clamp eps, inv=1/dn (DVE reciprocal),
    sims=dots*inv, then vector.max + max_index -> per-partition top-8
    candidate values + column indices.
  - Host: gather 8x128x8 candidates, divide by ||q|| (order-preserving),
    global top-5, inverse-square-distance weights, tiny centroid gather.

Environment workaround: this container's walrus build rejects any instruction
with more than one semaphore wait, and tensor_tensor_reduce (InstISA) fails
codegen entirely; see split_sync_waits() below and the scalar_tensor_tensor
choice above.
"""

import contextlib

import numpy as np

import concourse.bass as bass
import concourse.mybir as mybir
from concourse.tile import TileContext
from concourse.bass_utils import run_bass_kernel_spmd

N_CORES = 8
D = 768
N_ROWS = 500000
SHARD = N_ROWS // N_CORES   # 62500
G = 4                       # db rows per partition per full tile
P = 128
TILE_ROWS = P * G           # 512
N4 = SHARD // TILE_ROWS     # 122 full tiles
TAIL_START = N4 * TILE_ROWS  # 62464: tail tile covers rows 62464..62592 (G=1)
PAD_ROWS = TAIL_START + P   # 62592 (92 pad rows)
NCOLS = N4 * G + 1          # 489 candidate columns per partition
FREE = G * D                # 3072
K = 5
COS_EPS = 1e-8
W_EPS = 1e-6

_f32 = mybir.dt.float32
_u32 = mybir.dt.uint32

_wsplit_ctr = [0]


def split_sync_waits(nc):
    """Workaround for this container's walrus build: it rejects any instruction
    carrying more than ONE semaphore wait ("Too many sync wait commands" in
    setupSyncWait during codegen). Tile's scheduler freely attaches several
    waits to one instruction, so after TileContext scheduling we split them:
    every instruction keeps its last wait, and each extra wait is hoisted onto
    its own NoOp placed immediately before it in the same basic block (same
    engine, so program order preserves wait-before-execute semantics)."""
    for f in nc.m.functions:
        for b in f.blocks:
            needs_fix = any(
                getattr(i, "sync_info", None) is not None
                and i.sync_info.on_wait
                and len(i.sync_info.on_wait) > 1
                for i in b.instructions
            )
            if not needs_fix:
                continue
            new_insts = []
            for inst in b.instructions:
                si = getattr(inst, "sync_info", None)
                if si is not None and si.on_wait and len(si.on_wait) > 1:
                    waits = list(si.on_wait)
                    for w in waits[:-1]:
                        _wsplit_ctr[0] += 1
                        nop = mybir.InstNoOp(
                            name=f"WSPLIT-{_wsplit_ctr[0]}", ins=[], outs=[]
                        )
                        nop.engine = inst.engine
                        nop.sync_info = mybir.SyncInfo(on_wait=[w], on_update=[])
                        new_insts.append(nop)
                    inst.sync_info = mybir.SyncInfo(
                        on_wait=[waits[-1]], on_update=list(si.on_update or [])
                    )
                new_insts.append(inst)
            b.instructions[:] = new_insts
    return nc


def build_nc(n4: int | None = None, tail: bool = True, db_bufs: int = 4,
             repeat: int = 1, dve_sq_every: int = 16, g: int = G,
             two_rings: bool = False):
    """n4 full [128, g*768] tiles plus an optional [128, 768] tail tile.
    repeat>1 wraps the body in tc.For_i for on-device timing (one NEFF).
    dve_sq_every=k moves every k-th square op from ACT to DVE (0 = all ACT)."""
    tile_rows = P * g
    free = g * D
    if n4 is None:
        n4 = SHARD // tile_rows
    ncols = n4 * g + (1 if tail else 0)
    rows = n4 * tile_rows + (P if tail else 0)
    nc = bass.Bass()
    db = nc.dram_tensor("db", [rows * D], _f32, kind="ExternalInput")
    qrep = nc.dram_tensor("qrep", [P, D], _f32, kind="ExternalInput")
    outv = nc.dram_tensor("outv", [P, 8], _f32, kind="ExternalOutput")
    outi = nc.dram_tensor("outi", [P, 8], _u32, kind="ExternalOutput")

    with TileContext(nc) as tc:
        with (
            tc.tile_pool(name="persist", bufs=1) as persist,
            tc.tile_pool(name="dbp", bufs=db_bufs) as dbp,
            tc.tile_pool(name="dv", bufs=3) as dvp,
            tc.tile_pool(name="da", bufs=3) as dap,
        ):
            loop = tc.For_i(0, repeat, 1) if repeat > 1 else contextlib.nullcontext()
            with loop:
                qt = persist.tile([P, D], _f32, tag="qt")
                nc.sync.dma_start(qt[:], qrep[:])

                dots = persist.tile([P, ncols], _f32, tag="dots")
                n2 = persist.tile([P, ncols], _f32, tag="n2")

                def process(sb_ap, col):
                    tout = dvp.tile([P, D], _f32, tag="tout")
                    nc.vector.scalar_tensor_tensor(
                        out=tout[:],
                        in0=sb_ap,
                        scalar=0.0,
                        in1=qt[:],
                        op0=mybir.AluOpType.bypass,
                        op1=mybir.AluOpType.mult,
                        accum_out=dots[:, col : col + 1],
                    )
                    # ACT busy (~490us) runs close to the DMA roofline (~545us)
                    # while DVE has ~110us slack; shifting every 16th square to
                    # DVE balances both engines near ~460us.
                    if dve_sq_every and col % dve_sq_every == dve_sq_every - 1:
                        sq = dvp.tile([P, D], _f32, tag="tout")
                        nc.vector.scalar_tensor_tensor(
                            out=sq[:],
                            in0=sb_ap,
                            scalar=0.0,
                            in1=sb_ap,
                            op0=mybir.AluOpType.bypass,
                            op1=mybir.AluOpType.mult,
                            accum_out=n2[:, col : col + 1],
                        )
                    else:
                        aout = dap.tile([P, D], _f32, tag="aout")
                        nc.scalar.activation(
                            out=aout[:],
                            in_=sb_ap,
                            func=mybir.ActivationFunctionType.Square,
                            accum_out=n2[:, col : col + 1],
                        )

                # Tail tile first: keeps its DMA + compute off the end of the
                # critical path (the epilogue starts right after tile n4-1).
                if tail:
                    sbt = dbp.tile([P, free], _f32, tag="sb")
                    src = db[n4 * tile_rows * D : (n4 * tile_rows + P) * D]
                    nc.sync.dma_start(
                        sbt[:, :D], src.rearrange("(p f) -> p f", f=D)
                    )
                    process(sbt[:, :D], n4 * g)

                # sims = dots / max(sqrt(n2), eps), emitted in column chunks:
                # the first chunk only depends on the first half of the tiles,
                # so it overlaps with the second half of the DMA stream.
                dn = persist.tile([P, ncols], _f32, tag="dn")
                inv = persist.tile([P, ncols], _f32, tag="inv")
                sims = persist.tile([P, ncols], _f32, tag="sims")

                def epilogue_chunk(lo, hi):
                    if hi <= lo:
                        return
                    nc.scalar.sqrt(dn[:, lo:hi], n2[:, lo:hi])
                    nc.vector.tensor_scalar_max(dn[:, lo:hi], dn[:, lo:hi], COS_EPS)
                    nc.vector.reciprocal(inv[:, lo:hi], dn[:, lo:hi])
                    nc.vector.tensor_mul(sims[:, lo:hi], dots[:, lo:hi],
                                         inv[:, lo:hi])

                half_t = n4 // 2 + 1
                chunk0_hi = (n4 // 2) * g  # columns complete after tile half_t-1
                chunk0_done = False

                for t in range(n4):
                    sb = dbp.tile([P, free], _f32, tag="sb")
                    src = db[t * tile_rows * D : (t + 1) * tile_rows * D]
                    eng = nc.scalar if (two_rings and t % 2) else nc.sync
                    eng.dma_start(
                        sb[:], src.rearrange("(p f) -> p f", f=free)
                    )
                    for j in range(g):
                        process(sb[:, j * D : (j + 1) * D], t * g + j)
                    if t == half_t:
                        epilogue_chunk(0, chunk0_hi)
                        chunk0_done = True

                if not chunk0_done:
                    epilogue_chunk(0, chunk0_hi)
                epilogue_chunk(chunk0_hi, ncols)

                vals8 = persist.tile([P, 8], _f32, tag="vals8")
                idx8 = persist.tile([P, 8], _u32, tag="idx8")
                nc.vector.max(vals8[:], sims[:])
                nc.vector.max_index(idx8[:], vals8[:], sims[:])

                nc.sync.dma_start(outv[:], vals8[:])
                nc.sync.dma_start(outi[:], idx8[:])
    split_sync_waits(nc)
    return nc


def _prep_inputs(query: np.ndarray, database: np.ndarray, n_cores: int,
                 shard: int, n4: int | None = None, tail: bool = True,
                 g: int = G):
    """Build per-core input maps. Pads each shard with copies of -query
    (cosine similarity -1: never selected)."""
    tile_rows = P * g
    if n4 is None:
        n4 = shard // tile_rows if shard >= tile_rows else 0
    q = np.ascontiguousarray(np.asarray(query, dtype=np.float32)).reshape(1, D)
    db = np.asarray(database, dtype=np.float32)
    rows = n4 * tile_rows + (P if tail else 0)
    qrep = np.ascontiguousarray(np.tile(q, (P, 1)))  # [128, 768]
    in_maps = []
    for c in range(n_cores):
        sh = np.empty((rows, D), dtype=np.float32)
        sh[:shard] = db[c * shard : (c + 1) * shard]
        sh[shard:] = -q
        in_maps.append({"db": sh.reshape(-1), "qrep": qrep})
    return in_maps


def _host_reduce(results, query: np.ndarray, database: np.ndarray,
                 n_cores: int, shard: int, n4: int | None = None,
                 g: int = G) -> np.ndarray:
    tile_rows = P * g
    if n4 is None:
        n4 = shard // tile_rows if shard >= tile_rows else 0
    q = np.asarray(query, dtype=np.float32).reshape(1, D)
    db = np.asarray(database, dtype=np.float32)

    vals = np.stack([r["outv"] for r in results])          # [C,128,8] dot/||row||
    cols = np.stack([r["outi"] for r in results]).astype(np.int64)  # [C,128,8]

    c_idx = np.arange(n_cores, dtype=np.int64)[:, None, None]
    p_idx = np.arange(P, dtype=np.int64)[None, :, None]
    # cols < n4*g: full tile -> row = (col//g)*tile_rows + p*g + col%g
    # col == n4*g: tail tile -> row = n4*tile_rows + p
    t = cols // g
    j = cols % g
    shard_row = np.where(
        cols < n4 * g,
        t * tile_rows + p_idx * g + j,
        n4 * tile_rows + p_idx,
    )
    gidx = c_idx * shard + shard_row

    valid = (shard_row < shard).ravel()
    v = vals.ravel()[valid]
    g = gidx.ravel()[valid]

    qn = max(float(np.linalg.norm(q.astype(np.float64))), COS_EPS)
    sims = v / np.float32(qn)

    top = np.argsort(-sims, kind="stable")[:K]
    s = sims[top].astype(np.float64)
    idx = g[top]

    d = 1.0 - s
    w = 1.0 / (d + W_EPS) ** 2
    w = w / w.sum()
    centroid = (w[None, :] @ db[idx].astype(np.float64)).astype(np.float32)
    return centroid  # [1, D]


def _run(query: np.ndarray, database: np.ndarray, trace: bool = False):
    nc = build_nc()
    in_maps = _prep_inputs(query, database, N_CORES, SHARD)
    res = run_bass_kernel_spmd(
        nc, in_maps, core_ids=list(range(N_CORES)), trace=trace,
    )
    out = _host_reduce(res.results, query, database, N_CORES, SHARD)
    return out, res


def kernel(query: np.ndarray, database: np.ndarray) -> np.ndarray:
    out, _ = _run(query, database, trace=False)
    return out

